# revision 3
# baseline (speedup 1.0000x reference)
"""Trainium2 Bass kernel for top-2 MoE routing (nn_MoE_29291676959130).

Strategy: expert-parallel across the 8 NeuronCores (1 expert per core).
  host (shard step) : gating matmul in float64 -> top-2 indices + softmax
                      combine weights; gather each expert's routed tokens,
                      pre-scaled by their combine weight.
  device (per core) : Y = (w*x) @ W_e^T + w (x) b_e   for its <=CAP routed
                      tokens, as a float32r tiled matmul (K=2048 contraction
                      accumulated in PSUM, bias added as a rank-1 K=1 matmul).
  host (unshard)    : scatter-add the two expert contributions per token.

Shapes (hardcoded): B=4096 tokens, D=2048, H=4096, E=8 experts, top-2.
"""

import numpy as np

import concourse.bass as bass
import concourse.tile as tile
from concourse import bacc, mybir
from concourse.bass_utils import run_bass_kernel_spmd

B, D, H, E, TOPK = 4096, 2048, 4096, 8, 2
P = 128
KT = D // P          # 16 k-tiles over the contraction dim
NFREE = 512          # PSUM bank free-dim (fp32)
NT = H // NFREE      # 8 n-tiles

_NC_CACHE: dict[int, object] = {}

# Set TRACE=True (e.g. from a test harness) to profile the device kernel;
# the BassKernelResults of the last run lands in LAST_RESULTS.
TRACE = False
LAST_RESULTS = None


def _build(cap: int):
    """Per-core program: out[cap,H] = xt.T @ wt + wrow.T @ brow (float32r)."""
    fr = mybir.dt.float32r
    f32 = mybir.dt.float32
    nc = bacc.Bacc("TRN2", target_bir_lowering=False, debug=False, num_devices=E)
    xt = nc.dram_tensor("xt", [D, cap], fr, kind="ExternalInput").ap()
    wt = nc.dram_tensor("wt", [D, H], fr, kind="ExternalInput").ap()
    wrow = nc.dram_tensor("wrow", [1, cap], fr, kind="ExternalInput").ap()
    brow = nc.dram_tensor("brow", [1, H], fr, kind="ExternalInput").ap()
    out = nc.dram_tensor("out", [cap, H], f32, kind="ExternalOutput").ap()
    MT = cap // P

    with tile.TileContext(nc) as tc:
        with (
            tc.tile_pool(name="xpool", bufs=1) as xpool,
            tc.tile_pool(name="cpool", bufs=1) as cpool,
            tc.tile_pool(name="wpool", bufs=3) as wpool,
            tc.tile_pool(name="opool", bufs=4) as opool,
            tc.tile_pool(name="pspool", bufs=4, space="PSUM") as pspool,
        ):
            # Resident tokens: xts[p, k, c] = xt[k*128 + p, c]
            xts = xpool.tile([P, KT, cap], fr, name="xts")
            nc.sync.dma_start(xts[:], xt.rearrange("(kt p) c -> p kt c", p=P))
            wr = cpool.tile([1, cap], fr, name="wr", tag="wr")
            nc.sync.dma_start(wr[:], wrow[:])
            br = cpool.tile([1, H], fr, name="br", tag="br")
            nc.sync.dma_start(br[:], brow[:])

            for n in range(NT):
                ws = wt[:, n * NFREE:(n + 1) * NFREE]
                wtile = wpool.tile([P, KT, NFREE], fr, name="wtile")
                nc.sync.dma_start(wtile[:], ws.rearrange("(kt p) f -> p kt f", p=P))
                for m in range(MT):
                    acc = pspool.tile([P, NFREE], f32, name="acc")
                    for k in range(KT):
                        nc.tensor.matmul(
                            acc[:],
                            xts[:, k, m * P:(m + 1) * P],
                            wtile[:, k, :],
                            start=(k == 0),
                            stop=False,
                        )
                    # rank-1 bias term: += wrow[m-slice]^T @ brow[n-slice]
                    nc.tensor.matmul(
                        acc[:],
                        wr[:, m * P:(m + 1) * P],
                        br[:, n * NFREE:(n + 1) * NFREE],
                        start=False,
                        stop=True,
                    )
                    ot = opool.tile([P, NFREE], f32, name="ot")
                    nc.vector.tensor_copy(ot[:], acc[:])
                    nc.sync.dma_start(
                        out[m * P:(m + 1) * P, n * NFREE:(n + 1) * NFREE], ot[:]
                    )
    nc.compile()
    return nc


def _get_nc(cap: int):
    if cap not in _NC_CACHE:
        _NC_CACHE[cap] = _build(cap)
    return _NC_CACHE[cap]


def _route(x, difficulty_labels, emb, gate_W, gate_b):
    """Gating in float64: returns (topk_idx int32 [B,2], probs f64 [B,2])."""
    x64 = x.astype(np.float64)
    w1 = gate_W[:, :D].astype(np.float64)          # [E, D]
    w2 = gate_W[:, D:].astype(np.float64)          # [E, D]
    table = emb.astype(np.float64) @ w2.T          # [NDIFF, E]
    logits = x64 @ w1.T + table[difficulty_labels] + gate_b.astype(np.float64)
    # jax.lax.top_k order: descending value, ties -> lower index first
    topk_idx = np.argsort(-logits, axis=1, kind="stable")[:, :TOPK]
    topw = np.take_along_axis(logits, topk_idx, axis=1)
    ex = np.exp(topw - topw.max(axis=1, keepdims=True))
    probs = ex / ex.sum(axis=1, keepdims=True)
    return topk_idx.astype(np.int32), probs


def kernel(x, difficulty_labels, emb, gate_W, gate_b, expert_W, expert_b):
    x = np.asarray(x, dtype=np.float32)
    difficulty_labels = np.asarray(difficulty_labels)
    emb = np.asarray(emb, dtype=np.float32)
    gate_W = np.asarray(gate_W, dtype=np.float32)
    gate_b = np.asarray(gate_b, dtype=np.float32)
    expert_W = np.asarray(expert_W, dtype=np.float32)
    expert_b = np.asarray(expert_b, dtype=np.float32)

    topk_idx, probs = _route(x, difficulty_labels, emb, gate_W, gate_b)

    # Per-expert routed token lists + combine weights
    rows_per_e, w_per_e = [], []
    for e in range(E):
        hit = topk_idx == e                         # [B, 2]
        mask = hit.any(axis=1)
        rows = np.nonzero(mask)[0]
        w = np.where(hit[rows, 0], probs[rows, 0], probs[rows, 1])
        rows_per_e.append(rows)
        w_per_e.append(w.astype(np.float32))

    cap = max(P, int(-(-max(len(r) for r in rows_per_e) // P)) * P)
    nc = _get_nc(cap)

    in_maps = []
    for e in range(E):
        rows, w = rows_per_e[e], w_per_e[e]
        xs = np.zeros((cap, D), dtype=np.float32)
        xs[: len(rows)] = x[rows] * w[:, None]
        wrow = np.zeros((1, cap), dtype=np.float32)
        wrow[0, : len(rows)] = w
        in_maps.append(
            {
                "xt": np.ascontiguousarray(xs.T),
                "wt": np.ascontiguousarray(expert_W[e].T),
                "wrow": wrow,
                "brow": expert_b[e].reshape(1, H).astype(np.float32),
            }
        )

    res = run_bass_kernel_spmd(nc, in_maps, list(range(E)), trace=TRACE)
    global LAST_RESULTS
    LAST_RESULTS = res

    out = np.zeros((B, H), dtype=np.float32)
    for e in range(E):
        rows = rows_per_e[e]
        out[rows] += res.results[e]["out"][: len(rows)]
    return out, topk_idx


# revision 7
# speedup vs baseline: 1.1403x; 1.1403x over previous
"""Trainium2 Bass kernel for top-2 MoE routing (nn_MoE_29291676959130).

Strategy: expert-parallel across the 8 NeuronCores (1 expert per core).
  host (shard step) : gating matmul in float64 -> top-2 indices + softmax
                      combine weights; gather each expert's routed tokens,
                      pre-scaled by their combine weight.
  device (per core) : Y = (w*x) @ W_e^T + w (x) b_e   for its <=CAP routed
                      tokens, as a float16 tiled matmul (K=2048 contraction
                      accumulated in fp32 PSUM, bias added as a rank-1 K=1
                      matmul).
  host (unshard)    : scatter-add the two expert contributions per token.

Shapes (hardcoded): B=4096 tokens, D=2048, H=4096, E=8 experts, top-2.
"""

import numpy as np

import concourse.bass as bass
import concourse.tile as tile
from concourse import bacc, mybir
from concourse.bass_utils import run_bass_kernel_spmd

B, D, H, E, TOPK = 4096, 2048, 4096, 8, 2
P = 128
KT = D // P          # 16 k-tiles over the contraction dim
NFREE = 512          # PSUM bank free-dim (fp32)
NT = H // NFREE      # 8 n-tiles

_NC_CACHE: dict[int, object] = {}

# Set TRACE=True (e.g. from a test harness) to profile the device kernel;
# the BassKernelResults of the last run lands in LAST_RESULTS.
TRACE = False
LAST_RESULTS = None


def _build(cap: int):
    """Per-core program: out[cap,H] = xt.T @ wt + wrow.T @ brow (float16)."""
    fmm = mybir.dt.float16
    f32 = mybir.dt.float32
    nc = bacc.Bacc("TRN2", target_bir_lowering=False, debug=False, num_devices=E)
    xt = nc.dram_tensor("xt", [D, cap], fmm, kind="ExternalInput").ap()
    wt = nc.dram_tensor("wt", [D, H], fmm, kind="ExternalInput").ap()
    wrow = nc.dram_tensor("wrow", [1, cap], fmm, kind="ExternalInput").ap()
    brow = nc.dram_tensor("brow", [1, H], fmm, kind="ExternalInput").ap()
    out = nc.dram_tensor("out", [cap, H], f32, kind="ExternalOutput").ap()
    MT = cap // P

    with tile.TileContext(nc) as tc:
        with (
            tc.tile_pool(name="xpool", bufs=1) as xpool,
            tc.tile_pool(name="cpool", bufs=1) as cpool,
            tc.tile_pool(name="wpool", bufs=3) as wpool,
            tc.tile_pool(name="opool", bufs=4) as opool,
            tc.tile_pool(name="pspool", bufs=4, space="PSUM") as pspool,
        ):
            # Resident tokens: xts[p, k, c] = xt[k*128 + p, c].
            # Chunked DMA so the first matmuls start before the whole
            # token block has landed.
            xts = xpool.tile([P, KT, cap], fmm, name="xts")
            xt_r = xt.rearrange("(kt p) c -> p kt c", p=P)
            XCHUNK = 4
            for kc in range(0, KT, XCHUNK):
                nc.sync.dma_start(
                    xts[:, kc:kc + XCHUNK, :], xt_r[:, kc:kc + XCHUNK, :]
                )
            wr = cpool.tile([1, cap], fmm, name="wr", tag="wr")
            nc.sync.dma_start(wr[:], wrow[:])
            br = cpool.tile([1, H], fmm, name="br", tag="br")
            nc.sync.dma_start(br[:], brow[:])

            for n in range(NT):
                ws = wt[:, n * NFREE:(n + 1) * NFREE]
                wtile = wpool.tile([P, KT, NFREE], fmm, name="wtile")
                nc.sync.dma_start(wtile[:], ws.rearrange("(kt p) f -> p kt f", p=P))
                for m in range(MT):
                    acc = pspool.tile([P, NFREE], f32, name="acc")
                    for k in range(KT):
                        nc.tensor.matmul(
                            acc[:],
                            xts[:, k, m * P:(m + 1) * P],
                            wtile[:, k, :],
                            start=(k == 0),
                            stop=False,
                        )
                    # rank-1 bias term: += wrow[m-slice]^T @ brow[n-slice]
                    nc.tensor.matmul(
                        acc[:],
                        wr[:, m * P:(m + 1) * P],
                        br[:, n * NFREE:(n + 1) * NFREE],
                        start=False,
                        stop=True,
                    )
                    ot = opool.tile([P, NFREE], f32, name="ot")
                    nc.vector.tensor_copy(ot[:], acc[:])
                    nc.sync.dma_start(
                        out[m * P:(m + 1) * P, n * NFREE:(n + 1) * NFREE], ot[:]
                    )
    nc.compile()
    return nc


def _get_nc(cap: int):
    if cap not in _NC_CACHE:
        _NC_CACHE[cap] = _build(cap)
    return _NC_CACHE[cap]


def _route(x, difficulty_labels, emb, gate_W, gate_b):
    """Gating in float64: returns (topk_idx int32 [B,2], probs f64 [B,2])."""
    x64 = x.astype(np.float64)
    w1 = gate_W[:, :D].astype(np.float64)          # [E, D]
    w2 = gate_W[:, D:].astype(np.float64)          # [E, D]
    table = emb.astype(np.float64) @ w2.T          # [NDIFF, E]
    logits = x64 @ w1.T + table[difficulty_labels] + gate_b.astype(np.float64)
    # jax.lax.top_k order: descending value, ties -> lower index first
    topk_idx = np.argsort(-logits, axis=1, kind="stable")[:, :TOPK]
    topw = np.take_along_axis(logits, topk_idx, axis=1)
    ex = np.exp(topw - topw.max(axis=1, keepdims=True))
    probs = ex / ex.sum(axis=1, keepdims=True)
    return topk_idx.astype(np.int32), probs


def kernel(x, difficulty_labels, emb, gate_W, gate_b, expert_W, expert_b):
    x = np.asarray(x, dtype=np.float32)
    difficulty_labels = np.asarray(difficulty_labels)
    emb = np.asarray(emb, dtype=np.float32)
    gate_W = np.asarray(gate_W, dtype=np.float32)
    gate_b = np.asarray(gate_b, dtype=np.float32)
    expert_W = np.asarray(expert_W, dtype=np.float32)
    expert_b = np.asarray(expert_b, dtype=np.float32)

    topk_idx, probs = _route(x, difficulty_labels, emb, gate_W, gate_b)

    # Per-expert routed token lists + combine weights
    rows_per_e, w_per_e = [], []
    for e in range(E):
        hit = topk_idx == e                         # [B, 2]
        mask = hit.any(axis=1)
        rows = np.nonzero(mask)[0]
        w = np.where(hit[rows, 0], probs[rows, 0], probs[rows, 1])
        rows_per_e.append(rows)
        w_per_e.append(w.astype(np.float32))

    cap = max(P, int(-(-max(len(r) for r in rows_per_e) // P)) * P)
    nc = _get_nc(cap)

    in_maps = []
    for e in range(E):
        rows, w = rows_per_e[e], w_per_e[e]
        xs = np.zeros((cap, D), dtype=np.float32)
        xs[: len(rows)] = x[rows] * w[:, None]
        wrow = np.zeros((1, cap), dtype=np.float16)
        wrow[0, : len(rows)] = w
        in_maps.append(
            {
                "xt": xs.T.astype(np.float16),
                "wt": expert_W[e].T.astype(np.float16),
                "wrow": wrow,
                "brow": expert_b[e].reshape(1, H).astype(np.float16),
            }
        )

    res = run_bass_kernel_spmd(nc, in_maps, list(range(E)), trace=TRACE)
    global LAST_RESULTS
    LAST_RESULTS = res

    out = np.zeros((B, H), dtype=np.float32)
    for e in range(E):
        rows = rows_per_e[e]
        out[rows] += res.results[e]["out"][: len(rows)]
    return out, topk_idx
